# revision 1
# baseline (speedup 1.0000x reference)
"""AdaAT (per-channel affine grid transform + trilinear grid_sample) on 8
Trainium2 NeuronCores, pure data parallel over the batch.

Math notes (see reference):
  - The sampling grid's z coordinate is constant per channel f, so trilinear
    sampling factors into a fixed per-channel blend of two consecutive depth
    planes followed by a 2D bilinear sample (bilinear is linear in the image).
  - Sample positions are affine in the output pixel coords (u, v):
        ix = Ax*u + Bx*v + Cx,  iy = Ay*u + By*v + Cy
    with per-(batch, channel) coefficients derived from scale/angle/trans.
  - Zero-padding semantics of grid_sample are implemented by padding each
    blended plane into a 21x21 slab (1 left/top border, 2 right/bottom) and
    clamping the continuous coords to [0, 19]: clamped positions land on the
    zero border exactly when the true sample is out of range.

Device pipeline per core (512 batches x 20 channels = 10240 pairs):
  A. linear heads on TensorE/ACT -> per-pair affine coefficient table S
  B. z-blend planes -> padded bf16 slabs in DRAM
  C. per 128-pair tile: coords via TensorE (rank-3 matmul vs [u;v;1]),
     clamp via double-relu on ACT, floor/frac via mod on DVE,
     flat indices -> DRAM -> wrapped layout -> gpsimd indirect_copy fetches
     the 4 bilinear corners from 4 shifted slab copies -> DRAM bounce to
     pair-major -> bilinear lerp on DVE -> out.
"""

import sys
import types

import numpy as np

import concourse.bass as bass
import concourse.bacc as bacc
import concourse.mybir as mybir
import concourse.tile as tile
from concourse.bass_utils import run_bass_kernel_spmd

dt = mybir.dt
Alu = mybir.AluOpType
Act = mybir.ActivationFunctionType

PI = 3.14159
NCORES = 8
B = 4096
BSH = B // NCORES          # 512 batches per core
F = 20                     # channels == depth
H = 18
W = 18
PD = 256                   # para_code dim
NPAIR = BSH * F            # 10240, pair k = f*BSH + b
SLOTS = 336                # 16 q x 21 j; list pos i = q + 16*j; points = i < 324
SLAB = 441                 # 21 x 21 padded plane
SLABG = 442                # gather view size (4-byte aligned in bf16)
JW = 22                    # idx row width per octet (u16, 4-byte aligned)
GN = 352                   # gather list length = 16*JW (16 tail dummies)
SLABSTRIDE = 464
NPTS = H * W               # 324


def _blend_consts():
    """Per-channel (lo_plane, lo_w, hi_plane, hi_w) matching reference f32 math."""
    f = np.arange(F, dtype=np.float32)
    z = (np.float32(2.0) * f / np.float32(F - 1) - np.float32(1.0)).astype(np.float32)
    iz = ((z + np.float32(1.0)) * np.float32(F) - np.float32(1.0)) * np.float32(0.5)
    zlo = np.floor(iz)
    wz = (iz - zlo).astype(np.float32)
    out = []
    for i in range(F):
        lo = int(zlo[i])
        hi = lo + 1
        lo_w = float(1.0 - wz[i]) if 0 <= lo <= F - 1 else 0.0
        hi_w = float(wz[i]) if 0 <= hi <= F - 1 else 0.0
        out.append((max(lo, 0), lo_w, min(hi, F - 1), hi_w))
    return out


BLEND = _blend_consts()


def build_nc(bsh=BSH, debug=False):
    npair = bsh * F
    ntile = npair // 128
    nbt = (bsh + 127) // 128   # batch tiles for blend phase

    nc = bacc.Bacc()

    fm_d = nc.dram_tensor("fm", [bsh, F * 378], dt.float32, kind="ExternalInput")
    pcT_d = nc.dram_tensor("pcT", [PD, bsh], dt.float32, kind="ExternalInput")
    W1_d = nc.dram_tensor("W1", [PD, PD], dt.float32, kind="ExternalInput")
    b1_d = nc.dram_tensor("b1c", [PD, 1], dt.float32, kind="ExternalInput")
    Ws_d = nc.dram_tensor("Ws", [PD, F], dt.float32, kind="ExternalInput")
    bs_d = nc.dram_tensor("bsc", [F, 1], dt.float32, kind="ExternalInput")
    Wr_d = nc.dram_tensor("Wr", [PD, F], dt.float32, kind="ExternalInput")
    br_d = nc.dram_tensor("brc", [F, 1], dt.float32, kind="ExternalInput")
    Wt_d = nc.dram_tensor("Wt", [PD, 2 * F], dt.float32, kind="ExternalInput")
    bt_d = nc.dram_tensor("btc", [2 * F, 1], dt.float32, kind="ExternalInput")
    uv1_d = nc.dram_tensor("uv1", [3, SLOTS], dt.float32, kind="ExternalInput")
    ob_d = nc.dram_tensor("obias", [128, 1], dt.float32, kind="ExternalInput")

    out_d = nc.dram_tensor("out", [npair, SLOTS], dt.float32, kind="ExternalOutput")

    slabs_d = nc.dram_tensor("slabs_i", [bsh * F, 4, SLABSTRIDE], dt.bfloat16)
    sdram_d = nc.dram_tensor("sdram_i", [120, bsh], dt.float32)
    idxd_d = nc.dram_tensor("idxd_i", [npair, SLOTS], dt.uint16)
    corn_d = nc.dram_tensor("corn_i", [npair, 4 * SLOTS], dt.bfloat16)

    with tile.TileContext(nc) as tc:
        with tc.tile_pool(name="const", bufs=1) as constp:
            # ---------------- Phase A: heads -> S table ----------------
            pcT_sb = [constp.tile([128, bsh], dt.float32, tag=f"pcT{k}", name=f"pcT{k}") for k in range(2)]
            W1_sb = [constp.tile([128, PD], dt.float32, tag=f"W1{k}", name=f"W1s{k}") for k in range(2)]
            b1_sb = [constp.tile([128, 1], dt.float32, tag=f"b1{k}", name=f"b1s{k}") for k in range(2)]
            Ws_sb = [constp.tile([128, F], dt.float32, tag=f"Ws{k}", name=f"Wss{k}") for k in range(2)]
            Wr_sb = [constp.tile([128, F], dt.float32, tag=f"Wr{k}", name=f"Wrs{k}") for k in range(2)]
            Wt_sb = [constp.tile([128, 2 * F], dt.float32, tag=f"Wt{k}", name=f"Wts{k}") for k in range(2)]
            bs_sb = constp.tile([F, 1], dt.float32, tag="bs")
            br_sb = constp.tile([F, 1], dt.float32, tag="br")
            btx_sb = constp.tile([F, 1], dt.float32, tag="btx")
            bty_sb = constp.tile([F, 1], dt.float32, tag="bty")
            uv1_sb = constp.tile([3, SLOTS], dt.float32, tag="uv1")
            ob_sb = constp.tile([128, 1], dt.float32, tag="ob")

            for k in range(2):
                nc.sync.dma_start(out=pcT_sb[k][:], in_=pcT_d[128 * k:128 * (k + 1), :])
                nc.sync.dma_start(out=W1_sb[k][:], in_=W1_d[128 * k:128 * (k + 1), :])
                nc.sync.dma_start(out=b1_sb[k][:], in_=b1_d[128 * k:128 * (k + 1), :])
                nc.sync.dma_start(out=Ws_sb[k][:], in_=Ws_d[128 * k:128 * (k + 1), :])
                nc.sync.dma_start(out=Wr_sb[k][:], in_=Wr_d[128 * k:128 * (k + 1), :])
                nc.sync.dma_start(out=Wt_sb[k][:], in_=Wt_d[128 * k:128 * (k + 1), :])
            nc.sync.dma_start(out=bs_sb[:], in_=bs_d[:, :])
            nc.sync.dma_start(out=br_sb[:], in_=br_d[:, :])
            nc.sync.dma_start(out=btx_sb[:], in_=bt_d[0:F, :])
            nc.sync.dma_start(out=bty_sb[:], in_=bt_d[F:2 * F, :])
            nc.sync.dma_start(out=uv1_sb[:], in_=uv1_d[:, :])
            nc.sync.dma_start(out=ob_sb[:], in_=ob_d[:, :])

            # h^T = relu(W1^T @ pc^T + b1)  -> two [128, bsh] tiles
            hT = [constp.tile([128, bsh], dt.float32, tag=f"hT{m}", name=f"hTs{m}") for m in range(2)]
            sc01 = constp.tile([F, bsh], dt.float32, tag="sc01")
            ang = constp.tile([F, bsh], dt.float32, tag="ang")
            trx = constp.tile([F, bsh], dt.float32, tag="trx")
            try_ = constp.tile([F, bsh], dt.float32, tag="try_")
            with tc.tile_pool(name="head_ps", bufs=1, space="PSUM") as head_ps:
                for m in range(2):
                    ps = head_ps.tile([128, bsh], dt.float32, tag=f"hps{m}")
                    for k in range(2):
                        nc.tensor.matmul(ps[:], W1_sb[k][:, 128 * m:128 * (m + 1)],
                                         pcT_sb[k][:], start=(k == 0), stop=(k == 1))
                    nc.scalar.activation(hT[m][:], ps[:], Act.Relu, bias=b1_sb[m][:, 0:1])

                # heads: sigmoid / tanh / tanh
                ps_s = head_ps.tile([F, bsh], dt.float32, tag="ps_s")
                ps_r = head_ps.tile([F, bsh], dt.float32, tag="ps_r")
                ps_tx = head_ps.tile([F, bsh], dt.float32, tag="ps_tx")
                ps_ty = head_ps.tile([F, bsh], dt.float32, tag="ps_ty")
                for k in range(2):
                    nc.tensor.matmul(ps_s[:], Ws_sb[k][:], hT[k][:], start=(k == 0), stop=(k == 1))
                for k in range(2):
                    nc.tensor.matmul(ps_r[:], Wr_sb[k][:], hT[k][:], start=(k == 0), stop=(k == 1))
                for k in range(2):
                    nc.tensor.matmul(ps_tx[:], Wt_sb[k][:, 0:F], hT[k][:], start=(k == 0), stop=(k == 1))
                for k in range(2):
                    nc.tensor.matmul(ps_ty[:], Wt_sb[k][:, F:2 * F], hT[k][:], start=(k == 0), stop=(k == 1))
                nc.scalar.activation(sc01[:], ps_s[:], Act.Sigmoid, bias=bs_sb[:, 0:1])
                nc.scalar.activation(ang[:], ps_r[:], Act.Tanh, bias=br_sb[:, 0:1])
                nc.scalar.activation(trx[:], ps_tx[:], Act.Tanh, bias=btx_sb[:, 0:1])
                nc.scalar.activation(try_[:], ps_ty[:], Act.Tanh, bias=bty_sb[:, 0:1])

            # c = cos(PI*ang) = sin(wrap(PI*ang + pi/2)), s = sin(PI*ang)
            pt = constp.tile([F, bsh], dt.float32, tag="pt")
            carg = constp.tile([F, bsh], dt.float32, tag="carg")
            cosv = constp.tile([F, bsh], dt.float32, tag="cosv")
            sinv = constp.tile([F, bsh], dt.float32, tag="sinv")
            nc.vector.tensor_scalar(out=pt[:], in0=ang[:], scalar1=float(PI),
                                    scalar2=None, op0=Alu.mult)
            nc.vector.add_range_wrap(out=carg[:], in_=pt[:],
                                     shift=float(np.pi / 2), bound=float(np.pi),
                                     period=float(2 * np.pi))
            nc.scalar.activation(cosv[:], carg[:], Act.Sin)
            nc.scalar.activation(sinv[:], pt[:], Act.Sin)

            # per-quantity coefficient tiles (engine APs need 32-aligned bases)
            Ax_t = constp.tile([F, bsh], dt.float32, tag="Ax_t")
            Bx_t = constp.tile([F, bsh], dt.float32, tag="Bx_t")
            Cx_t = constp.tile([F, bsh], dt.float32, tag="Cx_t")
            Ay_t = constp.tile([F, bsh], dt.float32, tag="Ay_t")
            Cy_t = constp.tile([F, bsh], dt.float32, tag="Cy_t")
            p_ = constp.tile([F, bsh], dt.float32, tag="p_")
            q_ = constp.tile([F, bsh], dt.float32, tag="q_")
            d1 = constp.tile([F, bsh], dt.float32, tag="d1")
            e1 = constp.tile([F, bsh], dt.float32, tag="e1")
            t1 = constp.tile([F, bsh], dt.float32, tag="t1")
            t2 = constp.tile([F, bsh], dt.float32, tag="t2")
            nc.vector.tensor_tensor(out=p_[:], in0=sc01[:], in1=cosv[:], op=Alu.mult)
            nc.vector.tensor_tensor(out=q_[:], in0=sc01[:], in1=sinv[:], op=Alu.mult)
            K = 36.0 / 17.0
            nc.vector.tensor_scalar(out=Ax_t[:], in0=p_[:], scalar1=K,
                                    scalar2=None, op0=Alu.mult)
            nc.vector.tensor_scalar(out=Bx_t[:], in0=q_[:], scalar1=-K,
                                    scalar2=None, op0=Alu.mult)
            nc.vector.tensor_scalar(out=Ay_t[:], in0=q_[:], scalar1=K,
                                    scalar2=None, op0=Alu.mult)
            nc.vector.tensor_tensor(out=d1[:], in0=q_[:], in1=p_[:], op=Alu.subtract)
            nc.vector.tensor_tensor(out=e1[:], in0=q_[:], in1=p_[:], op=Alu.add)
            nc.vector.tensor_scalar(out=t1[:], in0=trx[:], scalar1=9.0, scalar2=9.5,
                                    op0=Alu.mult, op1=Alu.add)
            nc.vector.tensor_scalar(out=t2[:], in0=try_[:], scalar1=9.0, scalar2=9.5,
                                    op0=Alu.mult, op1=Alu.add)
            nc.vector.scalar_tensor_tensor(out=Cx_t[:], in0=d1[:], scalar=18.0,
                                           in1=t1[:], op0=Alu.mult, op1=Alu.add)
            nc.vector.scalar_tensor_tensor(out=Cy_t[:], in0=e1[:], scalar=-18.0,
                                           in1=t2[:], op0=Alu.mult, op1=Alu.add)

            # stage coefficient triples to DRAM; phase C reloads per-f
            # [3, bsh] slices at partition base 0 for TensorE
            sview2 = sdram_d[:].rearrange("(q f) b -> q f b", f=F)
            for qi, t in enumerate((Ax_t, Bx_t, Cx_t, Ay_t, Ax_t, Cy_t)):
                nc.sync.dma_start(out=sview2[qi], in_=t[:])

            # persistent gather-data buffers (junk partitions stay initialized),
            # allocated before the transient pools so addresses never overlap
            sdbuf = [constp.tile([128, 16 * SLABG], dt.bfloat16, tag=f"sdb{i}",
                                 name=f"sdb{i}") for i in range(3)]
            for t_ in sdbuf:
                nc.vector.memset(t_[:], 0.0)
            iwbuf = [constp.tile([128, 16 * JW], dt.uint16, tag=f"iwb{i}",
                                 name=f"iwb{i}") for i in range(3)]
            for t_ in iwbuf:
                nc.vector._memset_packed(t_[:], 0)

            # ---------------- Phase B: z-blend -> padded slabs ----------------
            # note: all pools stay open for the whole kernel; closing pools and
            # reusing their SBUF range trips released-zone sync imprecision.
            with (
                tc.tile_pool(name="fmp", bufs=1) as fmp,
                tc.tile_pool(name="slabsb", bufs=1) as slabsb,
                tc.tile_pool(name="coord_ps", bufs=2, space="PSUM") as coord_ps,
                tc.tile_pool(name="coords", bufs=3) as coords,
                tc.tile_pool(name="gath", bufs=2) as gath,
                tc.tile_pool(name="lerp", bufs=2) as lerpp,
            ):
                for bt4 in range(nbt):
                    b0 = 128 * bt4
                    bn = min(128, bsh - b0)
                    slab_sb = slabsb.tile([128, F * 464], dt.bfloat16, tag="slab")
                    nc.vector.memset(slab_sb[:], 0.0)
                    for half, (p0, p1, f0, f1) in enumerate(
                            ((0, 11, 0, 10), (9, 20, 10, 20))):
                        npl = p1 - p0
                        fm_sb = fmp.tile([128, 11 * 378], dt.float32, tag="fm")
                        nc.sync.dma_start(
                            out=fm_sb[:bn, 0:npl * 378],
                            in_=fm_d[b0:b0 + bn, p0 * 378:p1 * 378])
                        for f in range(f0, f1):
                            lo, lo_w, hi, hi_w = BLEND[f]
                            lo -= p0
                            hi -= p0
                            dst = slab_sb[:bn, f * 464 + 21:f * 464 + 21 + 378]
                            if lo_w == 0.0 or hi_w == 0.0:
                                pl, wgt = (hi, hi_w) if lo_w == 0.0 else (lo, lo_w)
                                nc.vector.tensor_scalar(
                                    out=dst, in0=fm_sb[:bn, pl * 378:(pl + 1) * 378],
                                    scalar1=float(wgt), scalar2=None, op0=Alu.mult)
                            else:
                                tf = fmp.tile([128, 378], dt.float32, tag="tf")
                                nc.scalar.activation(
                                    tf[:bn], fm_sb[:bn, hi * 378:(hi + 1) * 378],
                                    Act.Copy, scale=float(hi_w))
                                nc.vector.scalar_tensor_tensor(
                                    out=dst, in0=fm_sb[:bn, lo * 378:(lo + 1) * 378],
                                    scalar=float(lo_w), in1=tf[:bn],
                                    op0=Alu.mult, op1=Alu.add)
                    # store 4 shifted copies per slab: slab4[row, q, e] =
                    # slab[e + shift(q)], shift in (0, 1, 21, 22)
                    sb_flat = slab_sb[:bn]
                    sview4 = slabs_d[:].rearrange("(b f) q e -> b f q e", f=F)
                    for qv, shift in enumerate((0, 1, 21, 22)):
                        st_src = bass.AP(
                            tensor=sb_flat.tensor, offset=sb_flat.offset + shift,
                            ap=[list(sb_flat.ap[0]), [464, F], [1, SLABG]])
                        st_dst = sview4[b0:b0 + bn, :, qv, 0:SLABG]
                        eng = nc.scalar if qv % 2 else nc.sync
                        eng.dma_start(out=st_dst, in_=st_src)

                # ------------- Phase C: software-pipelined main loop -------------
                # stage A(T): coords + idx bounce + slab loads
                # stage B(T-1): gathers    stage C(T-2): corner bounce + lerp
                # Staggering keeps every engine's in-order stream from head-of-
                # line blocking on the previous tile's DMA round trips.
                NB = 3
                sflat = slabs_d[:].rearrange("r q e -> (r q e)")
                cflat = corn_d[:].rearrange("r e -> (r e)")
                st = {}

                def stage_a(T):
                    f = T // nbt
                    c4 = T % nbt
                    b0 = 128 * c4
                    if c4 == 0:
                        Sxf = coords.tile([3, bsh], dt.float32, tag="Sxf", name="Sxf")
                        Syf = coords.tile([3, bsh], dt.float32, tag="Syf", name="Syf")
                        nc.sync.dma_start(out=Sxf[:], in_=sview2[0:3, f, :])
                        nc.scalar.dma_start(out=Syf[:], in_=sview2[3:6, f, :])
                        st["Sxf"], st["Syf"] = Sxf, Syf
                    Sxf, Syf = st["Sxf"], st["Syf"]
                    ix_ps = coord_ps.tile([128, SLOTS], dt.float32, tag="ix_ps", name="ix_ps")
                    iy_ps = coord_ps.tile([128, SLOTS], dt.float32, tag="iy_ps", name="iy_ps")
                    nc.tensor.matmul(ix_ps[:], Sxf[:, b0:b0 + 128], uv1_sb[:],
                                     start=True, stop=True)
                    nc.tensor.matmul(iy_ps[:], Syf[:, b0:b0 + 128], uv1_sb[:],
                                     start=True, stop=True)
                    # xm = clamp(ix,0,19)-0.5; x0 = int(xm) is exact floor
                    # under any convert rounding mode (ties -> fx in {0,1})
                    txc = coords.tile([128, SLOTS], dt.float32, tag="txc", name="txc")
                    tyc = coords.tile([128, SLOTS], dt.float32, tag="tyc", name="tyc")
                    xm = coords.tile([128, SLOTS], dt.float32, tag="xm", name="xm")
                    ym = coords.tile([128, SLOTS], dt.float32, tag="ym", name="ym")
                    nc.scalar.activation(txc[:], ix_ps[:], Act.Copy, bias=-0.5)
                    nc.scalar.activation(tyc[:], iy_ps[:], Act.Copy, bias=-0.5)
                    nc.vector.tensor_scalar(out=xm[:], in0=txc[:], scalar1=-0.5,
                                            scalar2=18.5, op0=Alu.max, op1=Alu.min)
                    nc.vector.tensor_scalar(out=ym[:], in0=tyc[:], scalar1=-0.5,
                                            scalar2=18.5, op0=Alu.max, op1=Alu.min)
                    x0i = coords.tile([128, SLOTS], dt.int32, tag="txc", name="x0i")
                    y0i = coords.tile([128, SLOTS], dt.int32, tag="tyc", name="y0i")
                    x0f = coords.tile([128, SLOTS], dt.float32, tag="x0f", name="x0f")
                    y0f = coords.tile([128, SLOTS], dt.float32, tag="y0f", name="y0f")
                    nc.vector.tensor_copy(out=x0i[:], in_=xm[:])
                    nc.vector.tensor_copy(out=y0i[:], in_=ym[:])
                    nc.vector.tensor_copy(out=x0f[:], in_=x0i[:])
                    nc.vector.tensor_copy(out=y0f[:], in_=y0i[:])
                    fx = coords.tile([128, SLOTS], dt.bfloat16, tag="fx", name="fx")
                    fy = coords.tile([128, SLOTS], dt.bfloat16, tag="fy", name="fy")
                    nc.vector.scalar_tensor_tensor(out=fx[:], in0=xm[:], scalar=0.5,
                                                   in1=x0f[:], op0=Alu.add, op1=Alu.subtract)
                    nc.vector.scalar_tensor_tensor(out=fy[:], in0=ym[:], scalar=0.5,
                                                   in1=y0f[:], op0=Alu.add, op1=Alu.subtract)
                    st[("fx", T)] = fx
                    st[("fy", T)] = fy
                    idx_f = coords.tile([128, SLOTS], dt.float32, tag="xm", name="idx_f")
                    idx_s = coords.tile([128, SLOTS], dt.uint16, tag="idx_s", name="idx_s")
                    y0_s = y0f[:].rearrange("p (j q) -> p q j", q=16)
                    x0_s = x0f[:].rearrange("p (j q) -> p q j", q=16)
                    nc.vector.scalar_tensor_tensor(out=idx_f[:], in0=y0_s, scalar=21.0,
                                                   in1=x0_s, op0=Alu.mult, op1=Alu.add)
                    nc.vector.tensor_scalar(out=idx_s[:], in0=idx_f[:],
                                            scalar1=ob_sb[:, 0:1], scalar2=None,
                                            op0=Alu.add)
                    nc.sync.dma_start(out=idxd_d[128 * T:128 * (T + 1), :], in_=idx_s[:])
                    # wrapped idx reload
                    idxw = iwbuf[T % NB]
                    dstw = idxw[:].rearrange("p (o j) -> p o j", j=JW)[:, :, 0:21]
                    srcw = idxd_d[128 * T:128 * (T + 1), :].rearrange(
                        "(o g) (q j) -> (g q) o j", g=8, q=16)
                    nc.scalar.dma_start(out=dstw, in_=srcw)
                    # slab loads: one DMA per group g
                    slab_data = sdbuf[T % NB]
                    for g in range(8):
                        dstv = slab_data[16 * g:16 * g + 4, :].rearrange(
                            "q (o e) -> q o e", e=SLABG)
                        base = ((b0 + g) * F + f) * 4 * SLABSTRIDE
                        srcv = bass.AP(
                            tensor=sflat.tensor, offset=sflat.offset + base,
                            ap=[[SLABSTRIDE, 4],
                                [8 * F * 4 * SLABSTRIDE, 16], [1, SLABG]])
                        eng = nc.scalar if g % 2 else nc.sync
                        eng.dma_start(out=dstv, in_=srcv)

                def stage_b(T):
                    slab_data = sdbuf[T % NB]
                    idxw = iwbuf[T % NB]
                    gout = gath.tile([128, 16 * GN], dt.bfloat16, tag="gout", name="gout")
                    for o2 in range(16):
                        nc.gpsimd.indirect_copy(
                            out=gout[:, o2 * GN:(o2 + 1) * GN],
                            data=slab_data[:, o2 * SLABG:(o2 + 1) * SLABG],
                            idxs=idxw[:, o2 * JW:(o2 + 1) * JW],
                            i_know_ap_gather_is_preferred=True)
                    st[("gout", T)] = gout
                    # corner bounce out (pair-major in DRAM)
                    for g in range(8):
                        srcv = gout[16 * g:16 * g + 4, :].rearrange(
                            "q (o i) -> q o i", i=GN)[:, :, 0:SLOTS]
                        base = (128 * T + g) * 4 * SLOTS
                        dstv = bass.AP(
                            tensor=cflat.tensor, offset=cflat.offset + base,
                            ap=[[SLOTS, 4], [8 * 4 * SLOTS, 16], [1, SLOTS]])
                        eng = nc.sync if g % 2 else nc.scalar
                        eng.dma_start(out=dstv, in_=srcv)

                def stage_c(T):
                    st.pop(("gout", T), None)
                    fx = st.pop(("fx", T))
                    fy = st.pop(("fy", T))
                    cor = lerpp.tile([128, 4 * SLOTS], dt.bfloat16, tag="cor", name="cor")
                    nc.sync.dma_start(out=cor[:], in_=corn_d[128 * T:128 * (T + 1), :])
                    V00 = cor[:, 0:SLOTS]
                    V01 = cor[:, SLOTS:2 * SLOTS]
                    V10 = cor[:, 2 * SLOTS:3 * SLOTS]
                    V11 = cor[:, 3 * SLOTS:4 * SLOTS]
                    d0 = lerpp.tile([128, SLOTS], dt.bfloat16, tag="lt", name="d0")
                    vt = lerpp.tile([128, SLOTS], dt.bfloat16, tag="vt", name="vt")
                    vb = lerpp.tile([128, SLOTS], dt.bfloat16, tag="vb", name="vb")
                    res = lerpp.tile([128, SLOTS], dt.float32, tag="res", name="res")
                    nc.vector.tensor_tensor(out=d0[:], in0=V01, in1=V00, op=Alu.subtract)
                    nc.vector.tensor_tensor(out=d0[:], in0=d0[:], in1=fx[:], op=Alu.mult)
                    nc.vector.tensor_tensor(out=vt[:], in0=d0[:], in1=V00, op=Alu.add)
                    db = lerpp.tile([128, SLOTS], dt.bfloat16, tag="lt", name="db")
                    nc.vector.tensor_tensor(out=db[:], in0=V11, in1=V10, op=Alu.subtract)
                    nc.vector.tensor_tensor(out=db[:], in0=db[:], in1=fx[:], op=Alu.mult)
                    nc.vector.tensor_tensor(out=vb[:], in0=db[:], in1=V10, op=Alu.add)
                    dy = lerpp.tile([128, SLOTS], dt.bfloat16, tag="lt", name="dy")
                    nc.vector.tensor_tensor(out=dy[:], in0=vb[:], in1=vt[:], op=Alu.subtract)
                    nc.vector.tensor_tensor(out=dy[:], in0=dy[:], in1=fy[:], op=Alu.mult)
                    nc.vector.tensor_tensor(out=res[:], in0=dy[:], in1=vt[:], op=Alu.add)
                    nc.scalar.dma_start(out=out_d[128 * T:128 * (T + 1), :], in_=res[:])

                for step in range(ntile + 2):
                    if step < ntile:
                        stage_a(step)
                    if 0 <= step - 1 < ntile:
                        stage_b(step - 1)
                    if 0 <= step - 2 < ntile:
                        stage_c(step - 2)

    nc.compile()
    return nc


def _host_prep(feature_map, para_code, W1, b1, Ws, bs, Wr, br, Wt, bt, bsh=BSH):
    ncores = feature_map.shape[0] // bsh
    fm = np.pad(feature_map.astype(np.float32, copy=False),
                ((0, 0), (0, 0), (0, 0), (1, 2)))           # [B, F, 18, 21]
    fm = fm.reshape(ncores, bsh, F * 378)
    pcT = np.ascontiguousarray(
        para_code.astype(np.float32, copy=False).reshape(ncores, bsh, PD)
        .transpose(0, 2, 1))                                 # [ncores, 256, bsh]

    i = np.arange(SLOTS)
    u = np.where(i < NPTS, i % W, 0).astype(np.float32)
    v = np.where(i < NPTS, i // W, 0).astype(np.float32)
    uv1 = np.stack([u, v, np.ones(SLOTS, np.float32)])       # [3, 336]

    ob = np.zeros(128, np.float32)
    common = {
        "W1": np.ascontiguousarray(W1, np.float32),
        "b1c": np.ascontiguousarray(b1.reshape(PD, 1), np.float32),
        "Ws": np.ascontiguousarray(Ws, np.float32),
        "bsc": np.ascontiguousarray(bs.reshape(F, 1), np.float32),
        "Wr": np.ascontiguousarray(Wr, np.float32),
        "brc": np.ascontiguousarray(br.reshape(F, 1), np.float32),
        "Wt": np.ascontiguousarray(
            Wt.reshape(PD, F, 2).transpose(0, 2, 1).reshape(PD, 2 * F), np.float32),
        "btc": np.ascontiguousarray(
            bt.reshape(F, 2).T.reshape(2 * F, 1), np.float32),
        "uv1": np.ascontiguousarray(uv1),
        "obias": np.ascontiguousarray(ob.reshape(128, 1)),
    }
    in_maps = []
    for c in range(ncores):
        m = dict(common)
        m["fm"] = np.ascontiguousarray(fm[c])
        m["pcT"] = np.ascontiguousarray(pcT[c])
        in_maps.append(m)
    return in_maps


def _install_profile_hook():
    try:
        import antenv.axon_hooks  # noqa: F401
        return
    except ImportError:
        pass
    try:
        from trn_agent_boot.trn_boot import _ntff_profile_via_ctypes
        hook = _ntff_profile_via_ctypes("/opt/axon/libaxon_pjrt.so")
    except Exception:
        hook = None
    m = types.ModuleType("antenv.axon_hooks")
    m.get_axon_ntff_profile_hook = lambda: hook
    sys.modules["antenv.axon_hooks"] = m


_CACHED_NC = None


def kernel(feature_map, para_code, W1, b1, Ws, bs, Wr, br, Wt, bt,
           trace=False):
    global _CACHED_NC
    _install_profile_hook()
    if _CACHED_NC is None:
        _CACHED_NC = build_nc()
    nc = _CACHED_NC
    in_maps = _host_prep(feature_map, para_code, W1, b1, Ws, bs, Wr, br, Wt, bt)
    res = run_bass_kernel_spmd(nc, in_maps, core_ids=list(range(NCORES)),
                               trace=trace)
    outs = []
    for c in range(NCORES):
        o = res.results[c]["out"]                 # [NPAIR, 336] f32, pair k = f*BSH+b
        o = o[:, :NPTS].reshape(F, BSH, H, W).transpose(1, 0, 2, 3)
        outs.append(o)
    full = np.concatenate(outs, axis=0)
    kernel.last_exec_time_ns = getattr(res, "exec_time_ns", None)
    kernel.last_results = res
    return full



# revision 11
# speedup vs baseline: 1.0850x; 1.0850x over previous
"""AdaAT (per-channel affine grid transform + trilinear grid_sample) on 8
Trainium2 NeuronCores, pure data parallel over the batch.

Math notes (see reference):
  - The sampling grid's z coordinate is constant per channel f, so trilinear
    sampling factors into a fixed per-channel blend of two consecutive depth
    planes followed by a 2D bilinear sample (bilinear is linear in the image).
  - Sample positions are affine in the output pixel coords (u, v):
        ix = Ax*u + Bx*v + Cx,  iy = Ay*u + By*v + Cy
    with per-(batch, channel) coefficients derived from scale/angle/trans.
  - Zero-padding semantics of grid_sample are implemented by padding each
    blended plane into a 21x21 slab (1 left/top border, 2 right/bottom) and
    clamping the continuous coords to [0, 19]: clamped positions land on the
    zero border exactly when the true sample is out of range.

Device pipeline per core (512 batches x 20 channels = 10240 pairs):
  A. linear heads on TensorE/ACT -> per-pair affine coefficient table S
  B. z-blend planes -> padded bf16 slabs in DRAM
  C. per 128-pair tile: coords via TensorE (rank-3 matmul vs [u;v;1]),
     clamp via double-relu on ACT, floor/frac via mod on DVE,
     flat indices -> DRAM -> wrapped layout -> gpsimd indirect_copy fetches
     the 4 bilinear corners from 4 shifted slab copies -> DRAM bounce to
     pair-major -> bilinear lerp on DVE -> out.
"""

import sys
import types

import numpy as np

import concourse.bass as bass
import concourse.bacc as bacc
import concourse.mybir as mybir
import concourse.tile as tile
from concourse.bass_utils import run_bass_kernel_spmd

dt = mybir.dt
Alu = mybir.AluOpType
Act = mybir.ActivationFunctionType

PI = 3.14159
NCORES = 8
B = 4096
BSH = B // NCORES          # 512 batches per core
F = 20                     # channels == depth
H = 18
W = 18
PD = 256                   # para_code dim
NPAIR = BSH * F            # 10240, pair k = f*BSH + b
SLOTS = 336                # 16 q x 21 j; list pos i = q + 16*j; points = i < 324
SLAB = 441                 # 21 x 21 padded plane
SLABG = 442                # gather view size (4-byte aligned in bf16)
JW = 21                    # idx row width per octet (u16)
GN = 336                   # gather list length = 16*JW
ICB = 3                    # slab blocks per indirect_copy (3*336=1008 <= 1024)
SLABSTRIDE = 464
NPTS = H * W               # 324


def _blend_consts():
    """Per-channel (lo_plane, lo_w, hi_plane, hi_w) matching reference f32 math."""
    f = np.arange(F, dtype=np.float32)
    z = (np.float32(2.0) * f / np.float32(F - 1) - np.float32(1.0)).astype(np.float32)
    iz = ((z + np.float32(1.0)) * np.float32(F) - np.float32(1.0)) * np.float32(0.5)
    zlo = np.floor(iz)
    wz = (iz - zlo).astype(np.float32)
    out = []
    for i in range(F):
        lo = int(zlo[i])
        hi = lo + 1
        lo_w = float(1.0 - wz[i]) if 0 <= lo <= F - 1 else 0.0
        hi_w = float(wz[i]) if 0 <= hi <= F - 1 else 0.0
        out.append((max(lo, 0), lo_w, min(hi, F - 1), hi_w))
    return out


BLEND = _blend_consts()


def build_nc(bsh=BSH, debug=False):
    npair = bsh * F
    ntile = npair // 128
    nbt = (bsh + 127) // 128   # batch tiles for blend phase

    nc = bacc.Bacc()

    fm_d = nc.dram_tensor("fm", [bsh, F * 378], dt.float32, kind="ExternalInput")
    pcT_d = nc.dram_tensor("pcT", [PD, bsh], dt.float32, kind="ExternalInput")
    W1_d = nc.dram_tensor("W1", [PD, PD], dt.float32, kind="ExternalInput")
    b1_d = nc.dram_tensor("b1c", [PD, 1], dt.float32, kind="ExternalInput")
    Ws_d = nc.dram_tensor("Ws", [PD, F], dt.float32, kind="ExternalInput")
    bs_d = nc.dram_tensor("bsc", [F, 1], dt.float32, kind="ExternalInput")
    Wr_d = nc.dram_tensor("Wr", [PD, F], dt.float32, kind="ExternalInput")
    br_d = nc.dram_tensor("brc", [F, 1], dt.float32, kind="ExternalInput")
    Wt_d = nc.dram_tensor("Wt", [PD, 2 * F], dt.float32, kind="ExternalInput")
    bt_d = nc.dram_tensor("btc", [2 * F, 1], dt.float32, kind="ExternalInput")
    uv1_d = nc.dram_tensor("uv1", [3, SLOTS], dt.float32, kind="ExternalInput")
    ob_d = nc.dram_tensor("obias", [128, 1], dt.float32, kind="ExternalInput")

    out_d = nc.dram_tensor("out", [npair, SLOTS], dt.float32, kind="ExternalOutput")

    slabs_d = nc.dram_tensor("slabs_i", [bsh * F, 4, SLABSTRIDE], dt.bfloat16)
    sdram_d = nc.dram_tensor("sdram_i", [120, bsh], dt.float32)
    idxd_d = nc.dram_tensor("idxd_i", [npair, SLOTS], dt.uint16)
    corn_d = nc.dram_tensor("corn_i", [npair, 4 * SLOTS], dt.bfloat16)

    with tile.TileContext(nc) as tc:
        with tc.tile_pool(name="const", bufs=1) as constp:
            # ---------------- Phase A: heads -> S table ----------------
            pcT_sb = [constp.tile([128, bsh], dt.float32, tag=f"pcT{k}", name=f"pcT{k}") for k in range(2)]
            W1_sb = [constp.tile([128, PD], dt.float32, tag=f"W1{k}", name=f"W1s{k}") for k in range(2)]
            b1_sb = [constp.tile([128, 1], dt.float32, tag=f"b1{k}", name=f"b1s{k}") for k in range(2)]
            Ws_sb = [constp.tile([128, F], dt.float32, tag=f"Ws{k}", name=f"Wss{k}") for k in range(2)]
            Wr_sb = [constp.tile([128, F], dt.float32, tag=f"Wr{k}", name=f"Wrs{k}") for k in range(2)]
            Wt_sb = [constp.tile([128, 2 * F], dt.float32, tag=f"Wt{k}", name=f"Wts{k}") for k in range(2)]
            bs_sb = constp.tile([F, 1], dt.float32, tag="bs")
            br_sb = constp.tile([F, 1], dt.float32, tag="br")
            btx_sb = constp.tile([F, 1], dt.float32, tag="btx")
            bty_sb = constp.tile([F, 1], dt.float32, tag="bty")
            uv1_sb = constp.tile([3, SLOTS], dt.float32, tag="uv1")
            ob_sb = constp.tile([128, 1], dt.float32, tag="ob")

            for k in range(2):
                nc.sync.dma_start(out=pcT_sb[k][:], in_=pcT_d[128 * k:128 * (k + 1), :])
                nc.sync.dma_start(out=W1_sb[k][:], in_=W1_d[128 * k:128 * (k + 1), :])
                nc.sync.dma_start(out=b1_sb[k][:], in_=b1_d[128 * k:128 * (k + 1), :])
                nc.sync.dma_start(out=Ws_sb[k][:], in_=Ws_d[128 * k:128 * (k + 1), :])
                nc.sync.dma_start(out=Wr_sb[k][:], in_=Wr_d[128 * k:128 * (k + 1), :])
                nc.sync.dma_start(out=Wt_sb[k][:], in_=Wt_d[128 * k:128 * (k + 1), :])
            nc.sync.dma_start(out=bs_sb[:], in_=bs_d[:, :])
            nc.sync.dma_start(out=br_sb[:], in_=br_d[:, :])
            nc.sync.dma_start(out=btx_sb[:], in_=bt_d[0:F, :])
            nc.sync.dma_start(out=bty_sb[:], in_=bt_d[F:2 * F, :])
            nc.sync.dma_start(out=uv1_sb[:], in_=uv1_d[:, :])
            nc.sync.dma_start(out=ob_sb[:], in_=ob_d[:, :])

            # h^T = relu(W1^T @ pc^T + b1)  -> two [128, bsh] tiles
            hT = [constp.tile([128, bsh], dt.float32, tag=f"hT{m}", name=f"hTs{m}") for m in range(2)]
            sc01 = constp.tile([F, bsh], dt.float32, tag="sc01")
            ang = constp.tile([F, bsh], dt.float32, tag="ang")
            trx = constp.tile([F, bsh], dt.float32, tag="trx")
            try_ = constp.tile([F, bsh], dt.float32, tag="try_")
            with tc.tile_pool(name="head_ps", bufs=1, space="PSUM") as head_ps:
                for m in range(2):
                    ps = head_ps.tile([128, bsh], dt.float32, tag=f"hps{m}")
                    for k in range(2):
                        nc.tensor.matmul(ps[:], W1_sb[k][:, 128 * m:128 * (m + 1)],
                                         pcT_sb[k][:], start=(k == 0), stop=(k == 1))
                    nc.scalar.activation(hT[m][:], ps[:], Act.Relu, bias=b1_sb[m][:, 0:1])

                # heads: sigmoid / tanh / tanh
                ps_s = head_ps.tile([F, bsh], dt.float32, tag="ps_s")
                ps_r = head_ps.tile([F, bsh], dt.float32, tag="ps_r")
                ps_tx = head_ps.tile([F, bsh], dt.float32, tag="ps_tx")
                ps_ty = head_ps.tile([F, bsh], dt.float32, tag="ps_ty")
                for k in range(2):
                    nc.tensor.matmul(ps_s[:], Ws_sb[k][:], hT[k][:], start=(k == 0), stop=(k == 1))
                for k in range(2):
                    nc.tensor.matmul(ps_r[:], Wr_sb[k][:], hT[k][:], start=(k == 0), stop=(k == 1))
                for k in range(2):
                    nc.tensor.matmul(ps_tx[:], Wt_sb[k][:, 0:F], hT[k][:], start=(k == 0), stop=(k == 1))
                for k in range(2):
                    nc.tensor.matmul(ps_ty[:], Wt_sb[k][:, F:2 * F], hT[k][:], start=(k == 0), stop=(k == 1))
                nc.scalar.activation(sc01[:], ps_s[:], Act.Sigmoid, bias=bs_sb[:, 0:1])
                nc.scalar.activation(ang[:], ps_r[:], Act.Tanh, bias=br_sb[:, 0:1])
                nc.scalar.activation(trx[:], ps_tx[:], Act.Tanh, bias=btx_sb[:, 0:1])
                nc.scalar.activation(try_[:], ps_ty[:], Act.Tanh, bias=bty_sb[:, 0:1])

            # c = cos(PI*ang) = sin(wrap(PI*ang + pi/2)), s = sin(PI*ang)
            pt = constp.tile([F, bsh], dt.float32, tag="pt")
            carg = constp.tile([F, bsh], dt.float32, tag="carg")
            cosv = constp.tile([F, bsh], dt.float32, tag="cosv")
            sinv = constp.tile([F, bsh], dt.float32, tag="sinv")
            nc.vector.tensor_scalar(out=pt[:], in0=ang[:], scalar1=float(PI),
                                    scalar2=None, op0=Alu.mult)
            nc.vector.add_range_wrap(out=carg[:], in_=pt[:],
                                     shift=float(np.pi / 2), bound=float(np.pi),
                                     period=float(2 * np.pi))
            nc.scalar.activation(cosv[:], carg[:], Act.Sin)
            nc.scalar.activation(sinv[:], pt[:], Act.Sin)

            # per-quantity coefficient tiles (engine APs need 32-aligned bases)
            Ax_t = constp.tile([F, bsh], dt.float32, tag="Ax_t")
            Bx_t = constp.tile([F, bsh], dt.float32, tag="Bx_t")
            Cx_t = constp.tile([F, bsh], dt.float32, tag="Cx_t")
            Ay_t = constp.tile([F, bsh], dt.float32, tag="Ay_t")
            Cy_t = constp.tile([F, bsh], dt.float32, tag="Cy_t")
            p_ = constp.tile([F, bsh], dt.float32, tag="p_")
            q_ = constp.tile([F, bsh], dt.float32, tag="q_")
            d1 = constp.tile([F, bsh], dt.float32, tag="d1")
            e1 = constp.tile([F, bsh], dt.float32, tag="e1")
            t1 = constp.tile([F, bsh], dt.float32, tag="t1")
            t2 = constp.tile([F, bsh], dt.float32, tag="t2")
            nc.vector.tensor_tensor(out=p_[:], in0=sc01[:], in1=cosv[:], op=Alu.mult)
            nc.vector.tensor_tensor(out=q_[:], in0=sc01[:], in1=sinv[:], op=Alu.mult)
            K = 36.0 / 17.0
            nc.vector.tensor_scalar(out=Ax_t[:], in0=p_[:], scalar1=K,
                                    scalar2=None, op0=Alu.mult)
            nc.vector.tensor_scalar(out=Bx_t[:], in0=q_[:], scalar1=-K,
                                    scalar2=None, op0=Alu.mult)
            nc.vector.tensor_scalar(out=Ay_t[:], in0=q_[:], scalar1=K,
                                    scalar2=None, op0=Alu.mult)
            nc.vector.tensor_tensor(out=d1[:], in0=q_[:], in1=p_[:], op=Alu.subtract)
            nc.vector.tensor_tensor(out=e1[:], in0=q_[:], in1=p_[:], op=Alu.add)
            nc.vector.tensor_scalar(out=t1[:], in0=trx[:], scalar1=9.0, scalar2=9.5,
                                    op0=Alu.mult, op1=Alu.add)
            nc.vector.tensor_scalar(out=t2[:], in0=try_[:], scalar1=9.0, scalar2=9.5,
                                    op0=Alu.mult, op1=Alu.add)
            nc.vector.scalar_tensor_tensor(out=Cx_t[:], in0=d1[:], scalar=18.0,
                                           in1=t1[:], op0=Alu.mult, op1=Alu.add)
            nc.vector.scalar_tensor_tensor(out=Cy_t[:], in0=e1[:], scalar=-18.0,
                                           in1=t2[:], op0=Alu.mult, op1=Alu.add)

            # stage coefficient triples to DRAM; phase C reloads per-f
            # [3, bsh] slices at partition base 0 for TensorE
            sview2 = sdram_d[:].rearrange("(q f) b -> q f b", f=F)
            for qi, t in enumerate((Ax_t, Bx_t, Cx_t, Ay_t, Ax_t, Cy_t)):
                nc.sync.dma_start(out=sview2[qi], in_=t[:])

            # persistent gather-data buffers (junk partitions stay initialized),
            # allocated before the transient pools so addresses never overlap
            sdbuf = [constp.tile([128, 16 * SLABG], dt.bfloat16, tag=f"sdb{i}",
                                 name=f"sdb{i}") for i in range(3)]
            for t_ in sdbuf:
                nc.vector.memset(t_[:], 0.0)
            # idx tiles: one per gather chunk of ICB=3 blocks. Row = 3x21
            # entries + 1 dummy = 64 u16 (4B-aligned); the gather then emits
            # 16*64 = 1024 elems (the IC cap) of which the last 16 are junk
            # overwritten by the next chunk (chunks run ascending). The tail
            # chunk holds one block (22-wide row, 352 out).
            NCH = (16 + ICB - 1) // ICB
            CHW = [64, 64, 64, 64, 64, 22]
            iwbuf = [[constp.tile([128, CHW[c]], dt.uint16,
                                  tag=f"iwb{i}_{c}", name=f"iwb{i}_{c}")
                      for c in range(NCH)] for i in range(3)]
            for row in iwbuf:
                for t_ in row:
                    nc.vector._memset_packed(t_[:], 0)

            # ---------------- Phase B: z-blend -> padded slabs ----------------
            # note: all pools stay open for the whole kernel; closing pools and
            # reusing their SBUF range trips released-zone sync imprecision.
            with (
                tc.tile_pool(name="fmp", bufs=1) as fmp,
                tc.tile_pool(name="slabsb", bufs=1) as slabsb,
                tc.tile_pool(name="coord_ps", bufs=2, space="PSUM") as coord_ps,
                tc.tile_pool(name="coords", bufs=3) as coords,
                tc.tile_pool(name="gath", bufs=2) as gath,
                tc.tile_pool(name="lerp", bufs=2) as lerpp,
            ):
                for bt4 in range(nbt):
                    b0 = 128 * bt4
                    bn = min(128, bsh - b0)
                    slab_sb = slabsb.tile([128, F * 464], dt.bfloat16, tag="slab")
                    nc.vector.memset(slab_sb[:], 0.0)
                    for half, (p0, p1, f0, f1) in enumerate(
                            ((0, 11, 0, 10), (9, 20, 10, 20))):
                        npl = p1 - p0
                        fm_sb = fmp.tile([128, 11 * 378], dt.float32, tag="fm")
                        nc.sync.dma_start(
                            out=fm_sb[:bn, 0:npl * 378],
                            in_=fm_d[b0:b0 + bn, p0 * 378:p1 * 378])
                        for f in range(f0, f1):
                            lo, lo_w, hi, hi_w = BLEND[f]
                            lo -= p0
                            hi -= p0
                            dst = slab_sb[:bn, f * 464 + 21:f * 464 + 21 + 378]
                            if lo_w == 0.0 or hi_w == 0.0:
                                pl, wgt = (hi, hi_w) if lo_w == 0.0 else (lo, lo_w)
                                nc.vector.tensor_scalar(
                                    out=dst, in0=fm_sb[:bn, pl * 378:(pl + 1) * 378],
                                    scalar1=float(wgt), scalar2=None, op0=Alu.mult)
                            else:
                                tf = fmp.tile([128, 378], dt.float32, tag="tf")
                                nc.scalar.activation(
                                    tf[:bn], fm_sb[:bn, hi * 378:(hi + 1) * 378],
                                    Act.Copy, scale=float(hi_w))
                                nc.vector.scalar_tensor_tensor(
                                    out=dst, in0=fm_sb[:bn, lo * 378:(lo + 1) * 378],
                                    scalar=float(lo_w), in1=tf[:bn],
                                    op0=Alu.mult, op1=Alu.add)
                    # store 4 shifted copies per slab: slab4[row, q, e] =
                    # slab[e + shift(q)], shift in (0, 1, 21, 22)
                    sb_flat = slab_sb[:bn]
                    sview4 = slabs_d[:].rearrange("(b f) q e -> b f q e", f=F)
                    for qv, shift in enumerate((0, 1, 21, 22)):
                        st_src = bass.AP(
                            tensor=sb_flat.tensor, offset=sb_flat.offset + shift,
                            ap=[list(sb_flat.ap[0]), [464, F], [1, SLABG]])
                        st_dst = sview4[b0:b0 + bn, :, qv, 0:SLABG]
                        eng = nc.scalar if qv % 2 else nc.sync
                        eng.dma_start(out=st_dst, in_=st_src)

                # ------------- Phase C: software-pipelined main loop -------------
                # stage A(T): coords + idx bounce + slab loads
                # stage B(T-1): gathers    stage C(T-2): corner bounce + lerp
                # Staggering keeps every engine's in-order stream from head-of-
                # line blocking on the previous tile's DMA round trips.
                NB = 3
                sflat = slabs_d[:].rearrange("r q e -> (r q e)")
                cflat = corn_d[:].rearrange("r e -> (r e)")
                st = {}

                def stage_a(T):
                    f = T // nbt
                    c4 = T % nbt
                    b0 = 128 * c4
                    if c4 == 0:
                        Sxf = coords.tile([3, bsh], dt.float32, tag="Sxf", name="Sxf")
                        Syf = coords.tile([3, bsh], dt.float32, tag="Syf", name="Syf")
                        nc.sync.dma_start(out=Sxf[:], in_=sview2[0:3, f, :])
                        nc.scalar.dma_start(out=Syf[:], in_=sview2[3:6, f, :])
                        st["Sxf"], st["Syf"] = Sxf, Syf
                    Sxf, Syf = st["Sxf"], st["Syf"]
                    ix_ps = coord_ps.tile([128, SLOTS], dt.float32, tag="ix_ps", name="ix_ps")
                    iy_ps = coord_ps.tile([128, SLOTS], dt.float32, tag="iy_ps", name="iy_ps")
                    nc.tensor.matmul(ix_ps[:], Sxf[:, b0:b0 + 128], uv1_sb[:],
                                     start=True, stop=True)
                    nc.tensor.matmul(iy_ps[:], Syf[:, b0:b0 + 128], uv1_sb[:],
                                     start=True, stop=True)
                    # xm = clamp(ix,0,19)-0.5; x0 = int(xm) is exact floor
                    # under any convert rounding mode (ties -> fx in {0,1})
                    txc = coords.tile([128, SLOTS], dt.float32, tag="txc", name="txc")
                    tyc = coords.tile([128, SLOTS], dt.float32, tag="tyc", name="tyc")
                    xm = coords.tile([128, SLOTS], dt.float32, tag="xm", name="xm")
                    ym = coords.tile([128, SLOTS], dt.float32, tag="ym", name="ym")
                    nc.scalar.activation(txc[:], ix_ps[:], Act.Copy, bias=-0.5)
                    nc.scalar.activation(tyc[:], iy_ps[:], Act.Copy, bias=-0.5)
                    nc.vector.tensor_scalar(out=xm[:], in0=txc[:], scalar1=-0.5,
                                            scalar2=18.5, op0=Alu.max, op1=Alu.min)
                    nc.vector.tensor_scalar(out=ym[:], in0=tyc[:], scalar1=-0.5,
                                            scalar2=18.5, op0=Alu.max, op1=Alu.min)
                    x0i = coords.tile([128, SLOTS], dt.int32, tag="txc", name="x0i")
                    y0i = coords.tile([128, SLOTS], dt.int32, tag="tyc", name="y0i")
                    x0f = coords.tile([128, SLOTS], dt.float32, tag="x0f", name="x0f")
                    y0f = coords.tile([128, SLOTS], dt.float32, tag="y0f", name="y0f")
                    nc.vector.tensor_copy(out=x0i[:], in_=xm[:])
                    nc.vector.tensor_copy(out=y0i[:], in_=ym[:])
                    nc.vector.tensor_copy(out=x0f[:], in_=x0i[:])
                    nc.vector.tensor_copy(out=y0f[:], in_=y0i[:])
                    fx = coords.tile([128, SLOTS], dt.bfloat16, tag="fx", name="fx")
                    fy = coords.tile([128, SLOTS], dt.bfloat16, tag="fy", name="fy")
                    nc.vector.scalar_tensor_tensor(out=fx[:], in0=xm[:], scalar=0.5,
                                                   in1=x0f[:], op0=Alu.add, op1=Alu.subtract)
                    nc.vector.scalar_tensor_tensor(out=fy[:], in0=ym[:], scalar=0.5,
                                                   in1=y0f[:], op0=Alu.add, op1=Alu.subtract)
                    st[("fx", T)] = fx
                    st[("fy", T)] = fy
                    idx_f = coords.tile([128, SLOTS], dt.float32, tag="xm", name="idx_f")
                    idx_s = coords.tile([128, SLOTS], dt.uint16, tag="idx_s", name="idx_s")
                    y0_s = y0f[:].rearrange("p (j q) -> p q j", q=16)
                    x0_s = x0f[:].rearrange("p (j q) -> p q j", q=16)
                    nc.vector.scalar_tensor_tensor(out=idx_f[:], in0=y0_s, scalar=21.0,
                                                   in1=x0_s, op0=Alu.mult, op1=Alu.add)
                    nc.vector.tensor_scalar(out=idx_s[:], in0=idx_f[:],
                                            scalar1=ob_sb[:, 0:1], scalar2=None,
                                            op0=Alu.add)
                    nc.sync.dma_start(out=idxd_d[128 * T:128 * (T + 1), :], in_=idx_s[:])
                    # wrapped idx reload, one DMA per gather chunk
                    idxw = iwbuf[T % NB]
                    for c in range(NCH):
                        lo = c * ICB
                        nb = min(ICB, 16 - lo)
                        dstw = idxw[c][:, 0:nb * 21].rearrange(
                            "p (o j) -> p o j", j=21)
                        srcw = idxd_d[128 * T + 8 * lo:128 * T + 8 * (lo + nb),
                                      :].rearrange(
                            "(o g) (q j) -> (g q) o j", g=8, q=16)
                        eng = nc.scalar if c % 2 else nc.sync
                        eng.dma_start(out=dstw, in_=srcw)
                    # slab loads: one DMA per group g
                    slab_data = sdbuf[T % NB]
                    for g in range(8):
                        dstv = slab_data[16 * g:16 * g + 4, :].rearrange(
                            "q (o e) -> q o e", e=SLABG)
                        base = ((b0 + g) * F + f) * 4 * SLABSTRIDE
                        srcv = bass.AP(
                            tensor=sflat.tensor, offset=sflat.offset + base,
                            ap=[[SLABSTRIDE, 4],
                                [8 * F * 4 * SLABSTRIDE, 16], [1, SLABG]])
                        eng = nc.scalar if g % 2 else nc.sync
                        eng.dma_start(out=dstv, in_=srcv)

                def stage_b(T):
                    slab_data = sdbuf[T % NB]
                    idxw = iwbuf[T % NB]
                    # +16 tail cols absorb the final chunk's dummy-slot junk
                    gout = gath.tile([128, 16 * GN + 16], dt.bfloat16,
                                     tag="gout", name="gout")
                    # gathers carry a within-chunk block offset in the idx
                    # values (obias = (o%ICB)*442), so each instruction covers
                    # ICB sub-blocks: 16 -> 6 gpsimd dispatches per tile.
                    # Ascending order: each chunk's 16 junk cols (from the
                    # dummy 64th idx) are overwritten by the next chunk.
                    idxw = iwbuf[T % NB]
                    for c in range(NCH):
                        lo = c * ICB
                        nb = min(ICB, 16 - lo)
                        nc.gpsimd.indirect_copy(
                            out=gout[:, lo * GN:lo * GN + 16 * CHW[c]],
                            data=slab_data[:, lo * SLABG:(lo + nb) * SLABG],
                            idxs=idxw[c][:],
                            i_know_ap_gather_is_preferred=True)
                    st[("gout", T)] = gout
                    # corner bounce out (pair-major in DRAM)
                    for g in range(8):
                        srcv = gout[16 * g:16 * g + 4, 0:16 * GN].rearrange(
                            "q (o i) -> q o i", i=GN)[:, :, 0:SLOTS]
                        base = (128 * T + g) * 4 * SLOTS
                        dstv = bass.AP(
                            tensor=cflat.tensor, offset=cflat.offset + base,
                            ap=[[SLOTS, 4], [8 * 4 * SLOTS, 16], [1, SLOTS]])
                        eng = nc.sync if g % 2 else nc.scalar
                        eng.dma_start(out=dstv, in_=srcv)

                def stage_c(T):
                    st.pop(("gout", T), None)
                    fx = st.pop(("fx", T))
                    fy = st.pop(("fy", T))
                    cor = lerpp.tile([128, 4 * SLOTS], dt.bfloat16, tag="cor", name="cor")
                    nc.sync.dma_start(out=cor[:], in_=corn_d[128 * T:128 * (T + 1), :])
                    V00 = cor[:, 0:SLOTS]
                    V01 = cor[:, SLOTS:2 * SLOTS]
                    V10 = cor[:, 2 * SLOTS:3 * SLOTS]
                    V11 = cor[:, 3 * SLOTS:4 * SLOTS]
                    d0 = lerpp.tile([128, SLOTS], dt.bfloat16, tag="lt", name="d0")
                    vt = lerpp.tile([128, SLOTS], dt.bfloat16, tag="vt", name="vt")
                    vb = lerpp.tile([128, SLOTS], dt.bfloat16, tag="vb", name="vb")
                    res = lerpp.tile([128, SLOTS], dt.float32, tag="res", name="res")
                    nc.vector.tensor_tensor(out=d0[:], in0=V01, in1=V00, op=Alu.subtract)
                    nc.vector.tensor_tensor(out=d0[:], in0=d0[:], in1=fx[:], op=Alu.mult)
                    nc.vector.tensor_tensor(out=vt[:], in0=d0[:], in1=V00, op=Alu.add)
                    db = lerpp.tile([128, SLOTS], dt.bfloat16, tag="lt", name="db")
                    nc.vector.tensor_tensor(out=db[:], in0=V11, in1=V10, op=Alu.subtract)
                    nc.vector.tensor_tensor(out=db[:], in0=db[:], in1=fx[:], op=Alu.mult)
                    nc.vector.tensor_tensor(out=vb[:], in0=db[:], in1=V10, op=Alu.add)
                    dy = lerpp.tile([128, SLOTS], dt.bfloat16, tag="lt", name="dy")
                    nc.vector.tensor_tensor(out=dy[:], in0=vb[:], in1=vt[:], op=Alu.subtract)
                    nc.vector.tensor_tensor(out=dy[:], in0=dy[:], in1=fy[:], op=Alu.mult)
                    nc.vector.tensor_tensor(out=res[:], in0=dy[:], in1=vt[:], op=Alu.add)
                    nc.scalar.dma_start(out=out_d[128 * T:128 * (T + 1), :], in_=res[:])

                for step in range(ntile + 2):
                    if step < ntile:
                        stage_a(step)
                    if 0 <= step - 1 < ntile:
                        stage_b(step - 1)
                    if 0 <= step - 2 < ntile:
                        stage_c(step - 2)

    nc.compile()
    return nc


def _host_prep(feature_map, para_code, W1, b1, Ws, bs, Wr, br, Wt, bt, bsh=BSH):
    ncores = feature_map.shape[0] // bsh
    fm = np.pad(feature_map.astype(np.float32, copy=False),
                ((0, 0), (0, 0), (0, 0), (1, 2)))           # [B, F, 18, 21]
    fm = fm.reshape(ncores, bsh, F * 378)
    pcT = np.ascontiguousarray(
        para_code.astype(np.float32, copy=False).reshape(ncores, bsh, PD)
        .transpose(0, 2, 1))                                 # [ncores, 256, bsh]

    i = np.arange(SLOTS)
    u = np.where(i < NPTS, i % W, 0).astype(np.float32)
    v = np.where(i < NPTS, i // W, 0).astype(np.float32)
    uv1 = np.stack([u, v, np.ones(SLOTS, np.float32)])       # [3, 336]

    # per-partition index bias: pair p of a 128-tile lands in sub-block
    # o = p//8 of its gather group's data row; each indirect_copy covers
    # ICB consecutive sub-blocks, so bias by the offset within the chunk
    ob = (((np.arange(128) // 8) % ICB) * SLABG).astype(np.float32)
    common = {
        "W1": np.ascontiguousarray(W1, np.float32),
        "b1c": np.ascontiguousarray(b1.reshape(PD, 1), np.float32),
        "Ws": np.ascontiguousarray(Ws, np.float32),
        "bsc": np.ascontiguousarray(bs.reshape(F, 1), np.float32),
        "Wr": np.ascontiguousarray(Wr, np.float32),
        "brc": np.ascontiguousarray(br.reshape(F, 1), np.float32),
        "Wt": np.ascontiguousarray(
            Wt.reshape(PD, F, 2).transpose(0, 2, 1).reshape(PD, 2 * F), np.float32),
        "btc": np.ascontiguousarray(
            bt.reshape(F, 2).T.reshape(2 * F, 1), np.float32),
        "uv1": np.ascontiguousarray(uv1),
        "obias": np.ascontiguousarray(ob.reshape(128, 1)),
    }
    in_maps = []
    for c in range(ncores):
        m = dict(common)
        m["fm"] = np.ascontiguousarray(fm[c])
        m["pcT"] = np.ascontiguousarray(pcT[c])
        in_maps.append(m)
    return in_maps


def _install_profile_hook():
    try:
        import antenv.axon_hooks  # noqa: F401
        return
    except ImportError:
        pass
    try:
        from trn_agent_boot.trn_boot import _ntff_profile_via_ctypes
        hook = _ntff_profile_via_ctypes("/opt/axon/libaxon_pjrt.so")
    except Exception:
        hook = None
    m = types.ModuleType("antenv.axon_hooks")
    m.get_axon_ntff_profile_hook = lambda: hook
    sys.modules["antenv.axon_hooks"] = m


_CACHED_NC = None


def kernel(feature_map, para_code, W1, b1, Ws, bs, Wr, br, Wt, bt,
           trace=False):
    global _CACHED_NC
    _install_profile_hook()
    if _CACHED_NC is None:
        _CACHED_NC = build_nc()
    nc = _CACHED_NC
    in_maps = _host_prep(feature_map, para_code, W1, b1, Ws, bs, Wr, br, Wt, bt)
    res = run_bass_kernel_spmd(nc, in_maps, core_ids=list(range(NCORES)),
                               trace=trace)
    outs = []
    for c in range(NCORES):
        o = res.results[c]["out"]                 # [NPAIR, 336] f32, pair k = f*BSH+b
        o = o[:, :NPTS].reshape(F, BSH, H, W).transpose(1, 0, 2, 3)
        outs.append(o)
    full = np.concatenate(outs, axis=0)
    kernel.last_exec_time_ns = getattr(res, "exec_time_ns", None)
    kernel.last_results = res
    return full

